# revision 21
# baseline (speedup 1.0000x reference)
"""Multi-head attention (B=2, S=2048, D=2048, H=16) on 8 TRN2 NeuronCores.

Sharding: data-parallel over batch (2) x Megatron tensor-parallel over heads
(4 groups of 4 heads). Core c = 4*b + g handles batch b, heads [4g, 4g+4).
Each core computes q/k/v projections for its head slice, attention over its
4 heads, and a partial o_proj contribution; the host sums the 4 partials per
batch (the unshard step of Megatron TP) and stacks the 2 batches.

All matmuls run in float32r (TF32-like, ~12-bit mantissa, full PE rate);
softmax statistics and accumulations stay in fp32. Operands are pre-arranged
on the host into contraction-major (transposed) layouts and pre-rounded to
fp32r bit patterns; all FLOPs (projections, scores, softmax, attention,
o_proj) execute on device.

Device schedule: a K/V prefix (PE-bound projections), then per 512-query
chunk {q-projection, scores^T, exp (ACT), denominators (DVE+GPSIMD),
attn@v, o_proj} so PE matmul work overlaps the ACT/DVE/Pool-bound softmax.

HW exec time (8-core SPMD, For_i-calibrated): see test.py output.
"""

import math
import os

import numpy as np

import concourse.mybir as mybir
import concourse.tile as tile
from concourse import bacc
from concourse.bass_utils import run_bass_kernel_spmd

F32 = mybir.dt.float32
F32R = mybir.dt.float32r

B, S, D = 2, 2048, 2048
H = 16
HD = 128
G = 4
HLOC = H // G
DG = HLOC * HD
P = 128
NCORES = 8

SCHUNK = 512
NSC = S // SCHUNK
DT = D // P
ST_PER_CHUNK = SCHUNK // P
MT = DG // P
KT = S // P
QC = S // SCHUNK
IC = D // SCHUNK
HALF = SCHUNK // 2
INV_SQRT_HD = 1.0 / math.sqrt(HD)

_cache = {}
last_run = None  # BassKernelResults of the most recent execution (for test.py)


def build(loop_reps=None):
    nc = bacc.Bacc(None, target_bir_lowering=False)

    xT_dr = nc.dram_tensor("xT", [D, S], F32R, kind="ExternalInput")
    wqT_dr = nc.dram_tensor("wqT", [D, DG], F32R, kind="ExternalInput")
    wkT_dr = nc.dram_tensor("wkT", [D, DG], F32R, kind="ExternalInput")
    wvT_dr = nc.dram_tensor("wvT", [D, DG], F32R, kind="ExternalInput")
    woT_dr = nc.dram_tensor("woT", [DG, D], F32R, kind="ExternalInput")
    out_d = nc.dram_tensor("out", [S, D], F32, kind="ExternalOutput")

    # DRAM spill (per s-chunk for fine-grained deps); all writes/reads use
    # >=2KB contiguous lines
    qT_ds = [nc.dram_tensor(f"qT_s{i}", [DG, SCHUNK], F32R) for i in range(NSC)]
    v_ds = [nc.dram_tensor(f"v_s{i}", [SCHUNK, DG], F32R) for i in range(NSC)]
    xT_view = xT_dr.rearrange("(o p) s -> p o s", p=P)

    import contextlib

    with tile.TileContext(nc) as tc:
        loop_cm = tc.For_i(0, loop_reps, 1) if loop_reps else contextlib.nullcontext()
        with loop_cm:
            # kT is written by the prefix and read through the whole QA phase
            with tc.tile_pool(name="ktres", bufs=1) as ktpool:
                kT = ktpool.tile([P, HLOC, S], F32R, tag="kT")

                # ---------- prefix: k/q/v projections ----------
                with (
                    tc.tile_pool(name="wkvq", bufs=1) as wpool,
                    tc.tile_pool(name="xt", bufs=2) as xtpool,
                    tc.tile_pool(name="pstage", bufs=4) as pstage,
                    tc.tile_pool(name="psumP", bufs=1, space="PSUM") as psum,
                ):
                    wkT = wpool.tile([P, DT, DG], F32R, tag="wkT")
                    wqT = wpool.tile([P, DT, DG], F32R, tag="wqT")
                    wvT = wpool.tile([P, DT, DG], F32R, tag="wvT")
                    wkT_v = wkT_dr.rearrange("(o p) m -> p o m", p=P)
                    wqT_v = wqT_dr.rearrange("(o p) m -> p o m", p=P)
                    wvT_v = wvT_dr.rearrange("(o p) m -> p o m", p=P)

                    for d0 in range(0, DT, 4):
                        nc.sync.dma_start(wkT[:, d0:d0 + 4], wkT_v[:, d0:d0 + 4])
                    for d0 in range(0, DT, 4):
                        nc.sync.dma_start(wqT[:, d0:d0 + 4], wqT_v[:, d0:d0 + 4])
                    for d0 in range(0, DT, 4):
                        nc.sync.dma_start(wvT[:, d0:d0 + 4], wvT_v[:, d0:d0 + 4])

                    for sc in range(NSC):
                        for half in range(2):
                            xTh = xtpool.tile([P, DT, HALF], F32R, tag="xTh")
                            c0 = sc * SCHUNK + half * HALF
                            for d0 in range(0, DT, 4):
                                nc.sync.dma_start(
                                    xTh[:, d0:d0 + 4], xT_view[:, d0:d0 + 4, c0:c0 + HALF])

                            # k-pass and q-pass on this 256-wide half
                            for name, wT in (("k", wkT), ("q", wqT)):
                                for mt in range(MT):
                                    ps = psum.tile([P, HALF], F32, tag="kqpsum", bufs=4)
                                    for dt in range(DT):
                                        nc.tensor.matmul(
                                            ps[:], wT[:, dt, mt * P:(mt + 1) * P],
                                            xTh[:, dt, :],
                                            start=(dt == 0), stop=(dt == DT - 1))
                                    if name == "k":
                                        nc.vector.tensor_copy(
                                            kT[:, mt, c0:c0 + HALF], ps[:])
                                    else:
                                        sb = pstage.tile([P, HALF], F32R, tag="qstage")
                                        nc.vector.tensor_copy(sb[:], ps[:])
                                        nc.sync.dma_start(
                                            qT_ds[sc][mt * P:(mt + 1) * P,
                                                      half * HALF:(half + 1) * HALF],
                                            sb[:])

                            # v-pass: two s-tiles inside this half
                            for sti in range(2):
                                st = half * 2 + sti
                                ps = psum.tile([P, DG], F32, tag="vpsum", bufs=2)
                                for dt in range(DT):
                                    nc.tensor.matmul(
                                        ps[:], xTh[:, dt, sti * P:(sti + 1) * P],
                                        wvT[:, dt, :],
                                        start=(dt == 0), stop=(dt == DT - 1))
                                sb = pstage.tile([P, DG], F32R, tag="vstage")
                                nc.vector.tensor_copy(sb[:], ps[:])
                                nc.sync.dma_start(
                                    v_ds[sc][st * P:(st + 1) * P, :], sb[:])

                # ---------- QA phase ----------
                with (
                    tc.tile_pool(name="wo2", bufs=1) as wopool,
                    tc.tile_pool(name="vres", bufs=1) as vpool,
                    tc.tile_pool(name="qts", bufs=2) as qts,
                    tc.tile_pool(name="ctx", bufs=2) as ctxpool,
                    tc.tile_pool(name="asmall", bufs=2) as small,
                    tc.tile_pool(name="psumQA", bufs=1, space="PSUM") as psum,
                ):
                    vv = vpool.tile([P, NSC, ST_PER_CHUNK, HLOC, HD], F32R, tag="vv")
                    for sc in range(NSC):
                        nc.sync.dma_start(
                            vv[:, sc],
                            v_ds[sc].rearrange("(t p) (h n) -> p t h n", p=P, n=HD))
                    woT = wopool.tile([P, MT, D], F32R, tag="woT")
                    woT_v = woT_dr.rearrange("(o p) i -> p o i", p=P)
                    for j0 in range(MT):
                        nc.sync.dma_start(woT[:, j0:j0 + 1], woT_v[:, j0:j0 + 1])

                    ones_f = small.tile([P, 1], F32, tag="ones_f", bufs=1)
                    nc.vector.memset(ones_f[:], 1.0)

                    for qc in range(QC):
                        qTs = qts.tile([P, HLOC, SCHUNK], F32R, tag="qTs")
                        for mt in range(MT):
                            nc.sync.dma_start(
                                qTs[:, mt, :], qT_ds[qc][mt * P:(mt + 1) * P, :])

                        ctx = ctxpool.tile([P, G, SCHUNK], F32R, tag="ctx")
                        for h in range(HLOC):
                            acc = small.tile([P, SCHUNK], F32, tag="acc")
                            acc2 = small.tile([P, SCHUNK], F32, tag="acc2")
                            pso = psum.tile([P, SCHUNK], F32, tag="pso", bufs=2)
                            for kt in range(KT):
                                pss = psum.tile([P, SCHUNK], F32, tag="pss", bufs=3)
                                nc.tensor.matmul(
                                    pss[:], kT[:, h, kt * P:(kt + 1) * P], qTs[:, h, :],
                                    start=True, stop=True)
                                expP = small.tile([P, SCHUNK], F32R, tag="expP", bufs=5)
                                nc.scalar.activation(
                                    expP[:], pss[:], mybir.ActivationFunctionType.Exp,
                                    scale=INV_SQRT_HD)
                                expf = expP[:].bitcast(F32)
                                if kt == 0:
                                    nc.vector.tensor_copy(acc[:], expf)
                                elif kt == 1:
                                    nc.gpsimd.tensor_copy(acc2[:], expf)
                                elif kt % 2 == 0:
                                    nc.vector.tensor_add(acc[:], acc[:], expf)
                                else:
                                    nc.gpsimd.tensor_add(acc2[:], acc2[:], expf)
                                nc.tensor.matmul(
                                    pso[:],
                                    vv[:, kt // ST_PER_CHUNK, kt % ST_PER_CHUNK, h, :],
                                    expP[:],
                                    start=(kt == 0), stop=(kt == KT - 1))
                            nc.vector.tensor_add(acc[:], acc[:], acc2[:])
                            # colsum in plain fp32 (no rounding needed)
                            pssum = psum.tile([1, SCHUNK], F32, tag="pssum", bufs=1)
                            nc.tensor.matmul(
                                pssum[:], ones_f[:], acc[:], start=True, stop=True)
                            recip = small.tile([1, SCHUNK], F32, tag="recip")
                            nc.vector.reciprocal(recip[:], pssum[:])
                            rb = small.tile([P, SCHUNK], F32, tag="rb")
                            nc.gpsimd.partition_broadcast(rb[:], recip[:])
                            nc.vector.tensor_mul(ctx[:, h, :], pso[:], rb[:])

                        # fused o_proj for this chunk
                        for st in range(ST_PER_CHUNK):
                            stile = qc * ST_PER_CHUNK + st
                            for ic in range(IC):
                                ps = psum.tile([P, SCHUNK], F32, tag="opsum", bufs=2)
                                for jt in range(MT):
                                    nc.tensor.matmul(
                                        ps[:], ctx[:, jt, st * P:(st + 1) * P],
                                        woT[:, jt, ic * SCHUNK:(ic + 1) * SCHUNK],
                                        start=(jt == 0), stop=(jt == MT - 1))
                                ob = small.tile([P, SCHUNK], F32, tag="ostage")
                                nc.vector.tensor_copy(ob[:], ps[:])
                                nc.sync.dma_start(
                                    out_d[stile * P:(stile + 1) * P,
                                          ic * SCHUNK:(ic + 1) * SCHUNK],
                                    ob[:])

    nc.finalize()
    return nc

_build = build


def _round_f32r(a):
    """Round fp32 to fp32r bit patterns (round-to-nearest-even to 12 explicit
    mantissa bits, TF32-like) -- matches the hardware's own rounding."""
    u = np.ascontiguousarray(a, dtype=np.float32).view(np.uint32)
    keep = np.uint32(0xFFFFF000)
    half = np.uint32(0x7FF)
    lsb = (u >> np.uint32(12)) & np.uint32(1)
    return ((u + half + lsb) & keep).view(np.float32)


def kernel(hidden_states, wq, wk, wv, wo):
    global last_run
    if "nc" not in _cache:
        _cache["nc"] = build()
    nc = _cache["nc"]

    hidden_states = np.asarray(hidden_states, dtype=np.float32)
    wq = np.asarray(wq, dtype=np.float32)
    wk = np.asarray(wk, dtype=np.float32)
    wv = np.asarray(wv, dtype=np.float32)
    wo = np.asarray(wo, dtype=np.float32)

    xT = [_round_f32r(hidden_states[b].T) for b in range(B)]
    in_maps = []
    for c in range(NCORES):
        b, g = divmod(c, G)
        sl = slice(g * DG, (g + 1) * DG)
        in_maps.append({
            "xT": xT[b],
            "wqT": _round_f32r(wq[sl, :].T),
            "wkT": _round_f32r(wk[sl, :].T),
            "wvT": _round_f32r(wv[sl, :].T),
            "woT": _round_f32r(wo[:, sl].T),
        })

    trace = os.environ.get("BASSKERNEL_TRACE", "0") == "1"
    last_run = run_bass_kernel_spmd(
        nc, in_maps, core_ids=list(range(NCORES)), trace=trace)

    out = np.empty((B, S, D), dtype=np.float32)
    for b in range(B):
        acc = None
        for g in range(G):
            part = last_run.results[b * G + g]["out"]
            acc = part.copy() if acc is None else acc + part
        out[b] = acc
    return out
